# revision 36
# baseline (speedup 1.0000x reference)
"""Trainium2 Bass kernel for nn_ConceptEmbedding (type-conditioned embedding lookup).

Reference computation (per token position (b, s)):
    t = token_type[b, s]
    out[b, s, :] = proc_emb[concept]  if t == 1
                   med_emb[concept]   if t == 2
                   chart_emb[concept] if t == 3
                   0                  otherwise

Strategy (v11, trace-driven; baseline 48.9us -> 31.4us):
  - Fold the three tables into one [3V, E] table with flat row index
    (t-1)*V + concept. Tokens with t outside {1,2,3} produce zeros and are
    never sent to the device (the host assembles their rows as zeros).
  - Shard tokens across the 8 cores BY TABLE ROW RANGE: core c owns rows
    [c*37504, (c+1)*37504). The host hands each core a contiguous slice of
    the table ("twin") as its per-core input, so all gather windows have
    static bases. ~3072 typed tokens land on each core.
  - bf16 END TO END on the device: the host truncates the f32 table to
    bf16 (<=2^-8 relative error, budget is 2e-2) and upcasts the output.
    Halves every gathered and stored byte - decisive because DMA engine 15
    runs at ~60% of its peers and its serial packet queue is the critical
    path.
  - idx load issued by the Sync engine (HWDGE) as its first instruction so
    it overlaps the ~8.8us mlp library load; the library-reload pseudo-op
    is hoisted into the entry block (before the Bacc block barrier), which
    starts the async Q7 reload ~0.9us earlier.
  - OVERLAPPING gather windows: window A = twin[0:32768] (int16 idx covers
    rows < 32768), window B = twin[5504:38272] (idx = row-5504 covers
    everything above A). The host splits the ASCENDING-sorted row list by
    COUNT: the largest 512 rows go to the window-B gather, the rest
    (<=2688, all < 32768) fill three window-A queue contexts. Count-based
    splitting removes the statistical variance that would force fat caps.
  - desc-gen parallelism 4-wide (~7.1ns/desc async, ~14ns inline): queues
    1-3 gen on parallel contexts (two rounds each, 512 then 384 - the
    SWDGE pending ring only holds 2 un-generated instructions per queue)
    and the window-B gather gens INLINE on queue 0 as a 4th generator.
  - two HWDGE stores fire as soon as their gathers' DMA sems land,
    one descriptor per partition (small store packets trip a fixed
    per-packet penalty on DMA engine 15).
  - unused engines (PE/DVE/Activation) are culled from the compiled module
    (-3.6us of NRT launch/teardown handshaking).

dma_gather layout facts (verified on HW):
  - indices live at idxs[i % 16, i // 16], int16, replicated across all 128
    partitions; valid index i lands at dst[i % 128, i // 128, :].
  - one instruction must stay near ~1024 indices (the SWDGE descriptor ring
    is small; 1792-index gathers crash the exec unit).
  - the store view maps SBUF (p, block b) -> DRAM row p*NB + b, so the DRAM
    row for slot j of a gather at block base B0 is (j % 128)*NB + B0 + j//128.
"""

import numpy as np

V = 100000
E = 128
B = 16
S = 2048
NCORES = 8
P = 128

N_TOK = B * S  # 32768
NROWS = 3 * V  # 300000

RSPAN = 37504  # table rows owned per core (8 * 37504 >= 300000)
WB_BASE = 5504  # window B covers twin[5504:38272] (idx = row - 5504)
TWLEN = WB_BASE + 32768  # 38272 rows: window B must stay in-bounds

# Gather slots: window-B gather on queue 0 (inline gen, ~12ns/desc) plus two
# ROUNDS of window-A gathers on async queues 1-3 (~7.4ns/desc per context,
# ~1us fixed per instruction). Two rounds start the DMA-engine drain ~4us
# earlier than one big gather per context: DMA engine 15 runs gather packets
# at ~half speed, so its drain backlog is the transfer critical path and
# should start as early as possible.
# NOTE: the SWDGE pending ring holds only TWO un-generated instructions per
# queue - a third issue on the same queue stalls the gpsimd sequencer until
# the oldest finishes generating (cost v8 ~4us). Max 2 rounds per async queue.
CAP_B = 512
GATHERS = [  # (cap, slot0, window_base, queue) - slot order = SBUF block order
    # Per-queue context loads must be BALANCED: each context gens its two
    # rounds serially, so the makespan is the largest per-queue total
    # (staggering b caps across queues only unbalances it - measured +0.7us).
    # {768,128} rounds: same per-context total (balanced makespan) but the
    # SECOND round - the variance-exposed tail that the slowest DMA engine
    # drains last - carries only 98KB, and round-b gen ends ~0.9us earlier.
    (768, 0, 0, 1),  # round a: blocks 0-17
    (768, 768, 0, 2),
    (768, 1536, 0, 3),
    (128, 2304, 0, 1),  # round b: blocks 18-20
    (128, 2432, 0, 2),
    (128, 2560, 0, 3),
    (CAP_B, 2688, WB_BASE, 0),  # inline, blocks 21-24
]
CAPS_A = [c for c, _s, w, _q in GATHERS if w == 0]
SUMCAP = sum(c for c, _s, _w, _q in GATHERS)  # 3200
NB = SUMCAP // P  # 25 blocks
# Two stores, each one contiguous block range, one descriptor per partition
# (small store packets trip DMA engine 15's fixed per-packet penalty).
STORES = [((0, 1, 2), 0, 18), ((3, 4, 5, 6), 18, 25)]  # (ids, blk lo, hi)

_CACHED_NC = None


def _build_bass():
    global _CACHED_NC
    if _CACHED_NC is not None:
        return _CACHED_NC

    import concourse.bacc as bacc
    import concourse.mybir as mybir
    from concourse.library_config import mlp

    from contextlib import ExitStack

    # Raw Bacc Block (no Tile): explicit semaphores avoid Tile's multi-engine
    # teardown barrier cascade (~9us) and most of its sem-clear preamble.
    nc = bacc.Bacc(num_swdge_queues=4, monotonic_sem_count=0)
    twin = nc.dram_tensor("twin", [TWLEN, E], mybir.dt.bfloat16, kind="ExternalInput")
    idx = nc.dram_tensor("idx", [P, SUMCAP // 16], mybir.dt.int16, kind="ExternalInput")
    out = nc.dram_tensor("out", [SUMCAP, E], mybir.dt.bfloat16, kind="ExternalOutput")

    # SBUF (p, block b) <-> DRAM row p*NB + b
    out_v = out.rearrange("(p b) e -> p (b e)", p=P)

    with (
        ExitStack() as stack,
        nc.Block() as block,
        nc.sbuf_tensor("dst", [P, NB * E], mybir.dt.bfloat16) as dst,
        nc.sbuf_tensor("idxs", [P, SUMCAP // 16], mybir.dt.int16) as idxs,
        nc.semaphore("io") as io,
    ):
        # One shared semaphore per STORE group: the store then needs a single
        # wait (>= 16 * n_gathers) instead of a serially-decoded wait chain.
        ssems = [stack.enter_context(nc.semaphore(f"s{k}")) for k in range(len(STORES))]
        sem_of_gather = {}
        for sk, (gids, _blo, _bhi) in enumerate(STORES):
            for g in gids:
                sem_of_gather[g] = ssems[sk]

        @block.sync
        def _(sync):
            # idx load on HWDGE: overlaps the gpsimd library load.
            sync.dma_start(out=idxs[:], in_=idx[:]).then_inc(io, 16)
            # bf16 stores on HWDGE (everything is bf16 end-to-end, no cast);
            # each fires once its gather group's DMA-completion sems land.
            for sk, (gids, blo, bhi) in enumerate(STORES):
                sync.wait_ge(ssems[sk], 16 * len(gids))
                sync.dma_start(
                    out=out_v[:, blo * E : bhi * E],
                    in_=dst[:, blo * E : bhi * E],
                ).then_inc(io, 16)
            sync.wait_ge(io, 16 * (1 + len(STORES)))

        @block.gpsimd
        def _(gpsimd):
            gpsimd.load_library(mlp)
            gpsimd.wait_ge(io, 16)  # idx in SBUF
            # async queues first (issue returns fast), inline queue-0 last
            # (it blocks the engine for its whole gen).
            order = [k for k, g in enumerate(GATHERS) if g[3] != 0] + [
                k for k, g in enumerate(GATHERS) if g[3] == 0
            ]
            for k in order:
                cap, slot0, wb, qn = GATHERS[k]
                b0, bw = slot0 // P, cap // P
                in_ap = twin[wb : wb + 32768, :]
                d3 = dst[:, b0 * E : (b0 + bw) * E].rearrange("p (b e) -> p b e", e=E)
                gpsimd.dma_gather(
                    d3,
                    in_ap,
                    idxs[:, slot0 // 16 : (slot0 + cap) // 16],
                    cap,
                    cap,
                    E,
                    queue_num=qn,
                ).then_inc(sem_of_gather[k], 16)

    nc.finalize()
    if CULL_UNUSED_ENGINES:
        _cull_unused_engines(nc, mybir)
    _hoist_library_load(nc, mybir)
    _CACHED_NC = nc
    return nc


def _hoist_library_load(nc, mybir):
    """Move the Pool library-reload pseudo-instruction from the start of the
    gpsimd block into the entry block, after the Pool const-pool memsets but
    before the Pool-side block barrier. The Q7 reload (~8.8us) runs async to
    the sequencer, so hoisting it past the barrier exchange starts the load
    ~0.8us earlier without delaying the other engines' block entry."""
    import concourse.bass_isa as bass_isa

    entry = nc.main_func.blocks[0]
    reload_inst = None
    for b in nc.main_func.blocks[1:]:
        for inst in b.instructions:
            if isinstance(inst, bass_isa.InstPseudoReloadLibraryIndex):
                reload_inst = inst
                break
        if reload_inst is not None:
            if reload_inst.sync_info is not None:
                assert not reload_inst.sync_info.on_wait
            b.instructions.remove(reload_inst)
            break
    assert reload_inst is not None
    # insert after the last Pool InstMemset in the entry block
    pos = 0
    for i, inst in enumerate(entry.instructions):
        if (
            isinstance(inst, mybir.InstMemset)
            and inst.engine == mybir.EngineType.Pool
        ):
            pos = i + 1
    entry.instructions.insert(pos, reload_inst)


# Drop the Tensor/Vector/Scalar engines from the compiled module: the kernel
# only uses Pool (gpsimd) and SP (sync), but Bacc emits entry/exit barrier
# instructions for all five engines, and the NEFF then carries five engine
# queues whose NRT launch/teardown handshakes sit on the critical path.
# Removing the three idle engines' barrier legs (and shrinking the Pool-side
# barrier counts from 4 peers to 1) trims the fixed preamble/teardown cost.
CULL_UNUSED_ENGINES = True


def _cull_unused_engines(nc, mybir):
    cull = {
        mybir.EngineType.PE,
        mybir.EngineType.DVE,
        mybir.EngineType.Activation,
    }
    n_peers = 1  # SP is the only remaining non-Pool engine
    for b in nc.main_func.blocks:
        kept = [
            inst for inst in b.instructions if getattr(inst, "engine", None) not in cull
        ]
        if len(kept) != len(b.instructions):
            b.instructions[:] = kept
        for inst in kept:
            si = inst.sync_info
            if si is None:
                continue
            for w in si.on_wait:
                if "barrier" in (w.ant_name or "") and w.wait_value == 4:
                    w.wait_value = n_peers
            for u in si.on_update:
                if "barrier" in (u.ant_name or "") and u.update_value == 4:
                    u.update_value = n_peers


def _shard_inputs(proc_emb, med_emb, chart_emb, concept, token_type):
    """Returns (in_maps, plans, tables) with per-core slot bookkeeping."""
    import ml_dtypes

    tables = np.ascontiguousarray(
        np.concatenate(
            [
                np.asarray(proc_emb, dtype=np.float32),
                np.asarray(med_emb, dtype=np.float32),
                np.asarray(chart_emb, dtype=np.float32),
            ],
            axis=0,
        )
    )
    # bf16 table for the device: halves every gathered/stored byte. Truncation
    # (>>16) costs <= 2^-8 relative error - far inside the 2e-2 budget.
    tables_bf = (tables.view(np.uint32) >> 16).astype(np.uint16).view(
        ml_dtypes.bfloat16
    )
    tt = np.asarray(token_type).reshape(-1).astype(np.int64)
    cc = np.asarray(concept).reshape(-1).astype(np.int64)
    typed = (tt >= 1) & (tt <= 3)
    toks_all = np.where(typed)[0]  # global token ids with a real lookup
    eff = cc[toks_all] + (tt[toks_all] - 1) * V  # their table rows

    core_of = eff // RSPAN
    local = eff - core_of * RSPAN

    in_maps = []
    plans = []  # per core: (tokens, dram_rows, overflow_tokens, overflow_rows)
    for c in range(NCORES):
        base = c * RSPAN
        sl = tables_bf[base : min(base + TWLEN, NROWS)]
        if sl.shape[0] < TWLEN:
            sl = np.concatenate(
                [sl, np.zeros((TWLEN - sl.shape[0], E), ml_dtypes.bfloat16)]
            )
        twin = np.ascontiguousarray(sl)

        sel = np.where(core_of == c)[0]
        order = sel[np.argsort(local[sel], kind="stable")]
        lrows = local[order]  # ascending
        gtoks = toks_all[order]
        n = len(lrows)

        # Window-B gather takes the largest CAP_B rows (they must be >= 5504
        # to fit window B; everything >= 32768 MUST land there). Rest goes
        # ascending into the three window-A gathers (< 32768 required),
        # filled sequentially. Count-overflow tails are gathered on the host.
        ovf_toks, ovf_rows = [], []
        n_hi = int(n - np.searchsorted(lrows, 32768))  # rows >= 32768
        if n_hi > CAP_B:  # window-B capacity overflow -> host
            spill = n_hi - CAP_B
            ovf_toks.extend(gtoks[n - spill :].tolist())
            ovf_rows.extend(lrows[n - spill :].tolist())
            lrows, gtoks, n = lrows[:-spill], gtoks[:-spill], n - spill
            n_hi = CAP_B
        # fill window B up to CAP_B with the largest remaining rows >= WB_BASE
        n_b = min(CAP_B, n - int(np.searchsorted(lrows, WB_BASE)))
        n_a = n - n_b
        if n_a > sum(CAPS_A):  # window-A capacity overflow -> host
            spill = n_a - sum(CAPS_A)
            # drop the largest window-A rows to the host
            ovf_toks.extend(gtoks[n_a - spill : n_a].tolist())
            ovf_rows.extend(lrows[n_a - spill : n_a].tolist())
            lrows = np.concatenate([lrows[: n_a - spill], lrows[n_a:]])
            gtoks = np.concatenate([gtoks[: n_a - spill], gtoks[n_a:]])
            n, n_a = n - spill, sum(CAPS_A)

        # per-gather (lo, hi) in the sorted row list: the window-B gather
        # takes the tail [n_a:n]; window-A gathers fill sequentially.
        spans = []
        lo = 0
        for cap, _s, wb, _q in GATHERS:
            if wb == WB_BASE:
                spans.append((n_a, n))
            else:
                hi = min(lo + cap, n_a)
                spans.append((lo, hi))
                lo = hi
        idx16 = np.zeros((16, SUMCAP // 16), dtype=np.int16)
        tok_list, row_list = [], []
        for (cap, slot0, wb, _q), (lo, hi) in zip(GATHERS, spans):
            wrows, wtoks = lrows[lo:hi], gtoks[lo:hi]
            cnt = len(wrows)
            vals = np.zeros(cap, dtype=np.int16)
            vals[:cnt] = (wrows - wb).astype(np.int16)  # pad 0 = benign row
            idx16[:, slot0 // 16 : (slot0 + cap) // 16] = (
                vals.reshape(cap // 16, 16).T
            )
            j = np.arange(cnt)
            row_list.append((j % P) * NB + slot0 // P + j // P)
            tok_list.append(wtoks)

        in_maps.append(
            {"twin": twin, "idx": np.ascontiguousarray(np.tile(idx16, (8, 1)))}
        )
        plans.append(
            (
                np.concatenate(tok_list),
                np.concatenate(row_list),
                np.array(ovf_toks, dtype=np.int64),
                np.array(ovf_rows, dtype=np.int64) + base,
            )
        )

    return in_maps, plans, tables


def _run(in_maps, trace=False):
    from concourse.bass_utils import run_bass_kernel_spmd

    nc = _build_bass()
    return run_bass_kernel_spmd(nc, in_maps, list(range(NCORES)), trace=trace)


def _assemble(results, plans, tables):
    out = np.zeros((N_TOK, E), dtype=np.float32)
    for c in range(NCORES):
        toks, drows, ovf_toks, ovf_rows = plans[c]
        if len(toks):
            out[toks] = results[c]["out"][drows].astype(np.float32)
        if len(ovf_toks):
            out[ovf_toks] = tables[ovf_rows]
    return out.reshape(B, S, E)


def kernel(proc_emb, med_emb, chart_emb, concept, token_type):
    in_maps, plans, tables = _shard_inputs(
        proc_emb, med_emb, chart_emb, concept, token_type
    )
    res = _run(in_maps, trace=False)
    return _assemble(res.results, plans, tables)


# revision 37
# speedup vs baseline: 1.0941x; 1.0941x over previous
"""Trainium2 Bass kernel for nn_ConceptEmbedding (type-conditioned embedding lookup).

Reference computation (per token position (b, s)):
    t = token_type[b, s]
    out[b, s, :] = proc_emb[concept]  if t == 1
                   med_emb[concept]   if t == 2
                   chart_emb[concept] if t == 3
                   0                  otherwise

Strategy (v11, trace-driven; baseline 48.9us -> 31.4us):
  - Fold the three tables into one [3V, E] table with flat row index
    (t-1)*V + concept. Tokens with t outside {1,2,3} produce zeros and are
    never sent to the device (the host assembles their rows as zeros).
  - Shard tokens across the 8 cores BY TABLE ROW RANGE: core c owns rows
    [c*37504, (c+1)*37504). The host hands each core a contiguous slice of
    the table ("twin") as its per-core input, so all gather windows have
    static bases. ~3072 typed tokens land on each core.
  - bf16 END TO END on the device: the host truncates the f32 table to
    bf16 (<=2^-8 relative error, budget is 2e-2) and upcasts the output.
    Halves every gathered and stored byte - decisive because DMA engine 15
    runs at ~60% of its peers and its serial packet queue is the critical
    path.
  - idx load issued by the Sync engine (HWDGE) as its first instruction so
    it overlaps the ~8.8us mlp library load; the library-reload pseudo-op
    is hoisted into the entry block (before the Bacc block barrier), which
    starts the async Q7 reload ~0.9us earlier.
  - OVERLAPPING gather windows: window A = twin[0:32768] (int16 idx covers
    rows < 32768), window B = twin[5504:38272] (idx = row-5504 covers
    everything above A). The host splits the ASCENDING-sorted row list by
    COUNT: the largest 512 rows go to the window-B gather, the rest
    (<=2688, all < 32768) fill three window-A queue contexts. Count-based
    splitting removes the statistical variance that would force fat caps.
  - desc-gen parallelism 4-wide (~7.1ns/desc async, ~14ns inline): queues
    1-3 gen on parallel contexts (two rounds each, 512 then 384 - the
    SWDGE pending ring only holds 2 un-generated instructions per queue)
    and the window-B gather gens INLINE on queue 0 as a 4th generator.
  - two HWDGE stores fire as soon as their gathers' DMA sems land,
    one descriptor per partition (small store packets trip a fixed
    per-packet penalty on DMA engine 15).
  - unused engines (PE/DVE/Activation) are culled from the compiled module
    (-3.6us of NRT launch/teardown handshaking).

dma_gather layout facts (verified on HW):
  - indices live at idxs[i % 16, i // 16], int16, replicated across all 128
    partitions; valid index i lands at dst[i % 128, i // 128, :].
  - one instruction must stay near ~1024 indices (the SWDGE descriptor ring
    is small; 1792-index gathers crash the exec unit).
  - the store view maps SBUF (p, block b) -> DRAM row p*NB + b, so the DRAM
    row for slot j of a gather at block base B0 is (j % 128)*NB + B0 + j//128.
"""

import numpy as np

V = 100000
E = 128
B = 16
S = 2048
NCORES = 8
P = 128

N_TOK = B * S  # 32768
NROWS = 3 * V  # 300000

RSPAN = 37504  # table rows owned per core (8 * 37504 >= 300000)
WB_BASE = 5504  # window B covers twin[5504:38272] (idx = row - 5504)
TWLEN = WB_BASE + 32768  # 38272 rows: window B must stay in-bounds

# Gather slots: window-B gather on queue 0 (inline gen, ~12ns/desc) plus two
# ROUNDS of window-A gathers on async queues 1-3 (~7.4ns/desc per context,
# ~1us fixed per instruction). Two rounds start the DMA-engine drain ~4us
# earlier than one big gather per context: DMA engine 15 runs gather packets
# at ~half speed, so its drain backlog is the transfer critical path and
# should start as early as possible.
# NOTE: the SWDGE pending ring holds only TWO un-generated instructions per
# queue - a third issue on the same queue stalls the gpsimd sequencer until
# the oldest finishes generating (cost v8 ~4us). Max 2 rounds per async queue.
CAP_B = 512
GATHERS = [  # (cap, slot0, window_base, queue) - slot order = SBUF block order
    # Per-queue context loads must be BALANCED: each context gens its two
    # rounds serially, so the makespan is the largest per-queue total
    # (staggering b caps across queues only unbalances it - measured +0.7us).
    # {640,256} rounds: same per-context total (balanced makespan) but the
    # SECOND round - the variance-exposed tail that the slowest DMA engine
    # drains last - carries 196KB instead of 295KB.
    (640, 0, 0, 1),  # round a: blocks 0-14
    (640, 640, 0, 2),
    (640, 1280, 0, 3),
    (256, 1920, 0, 1),  # round b: blocks 15-20
    (256, 2176, 0, 2),
    (256, 2432, 0, 3),
    (CAP_B, 2688, WB_BASE, 0),  # inline, blocks 21-24
]
CAPS_A = [c for c, _s, w, _q in GATHERS if w == 0]
SUMCAP = sum(c for c, _s, _w, _q in GATHERS)  # 3200
NB = SUMCAP // P  # 25 blocks
# Two stores, each one contiguous block range, one descriptor per partition
# (small store packets trip DMA engine 15's fixed per-packet penalty).
STORES = [((0, 1, 2), 0, 15), ((3, 4, 5, 6), 15, 25)]  # (ids, blk lo, hi)

_CACHED_NC = None


def _build_bass():
    global _CACHED_NC
    if _CACHED_NC is not None:
        return _CACHED_NC

    import concourse.bacc as bacc
    import concourse.mybir as mybir
    from concourse.library_config import mlp

    from contextlib import ExitStack

    # Raw Bacc Block (no Tile): explicit semaphores avoid Tile's multi-engine
    # teardown barrier cascade (~9us) and most of its sem-clear preamble.
    nc = bacc.Bacc(num_swdge_queues=4, monotonic_sem_count=0)
    twin = nc.dram_tensor("twin", [TWLEN, E], mybir.dt.bfloat16, kind="ExternalInput")
    idx = nc.dram_tensor("idx", [P, SUMCAP // 16], mybir.dt.int16, kind="ExternalInput")
    out = nc.dram_tensor("out", [SUMCAP, E], mybir.dt.bfloat16, kind="ExternalOutput")

    # SBUF (p, block b) <-> DRAM row p*NB + b
    out_v = out.rearrange("(p b) e -> p (b e)", p=P)

    with (
        ExitStack() as stack,
        nc.Block() as block,
        nc.sbuf_tensor("dst", [P, NB * E], mybir.dt.bfloat16) as dst,
        nc.sbuf_tensor("idxs", [P, SUMCAP // 16], mybir.dt.int16) as idxs,
        nc.semaphore("io") as io,
    ):
        # One shared semaphore per STORE group: the store then needs a single
        # wait (>= 16 * n_gathers) instead of a serially-decoded wait chain.
        ssems = [stack.enter_context(nc.semaphore(f"s{k}")) for k in range(len(STORES))]
        sem_of_gather = {}
        for sk, (gids, _blo, _bhi) in enumerate(STORES):
            for g in gids:
                sem_of_gather[g] = ssems[sk]

        @block.sync
        def _(sync):
            # idx load on HWDGE: overlaps the gpsimd library load.
            sync.dma_start(out=idxs[:], in_=idx[:]).then_inc(io, 16)
            # bf16 stores on HWDGE (everything is bf16 end-to-end, no cast);
            # each fires once its gather group's DMA-completion sems land.
            for sk, (gids, blo, bhi) in enumerate(STORES):
                sync.wait_ge(ssems[sk], 16 * len(gids))
                sync.dma_start(
                    out=out_v[:, blo * E : bhi * E],
                    in_=dst[:, blo * E : bhi * E],
                ).then_inc(io, 16)
            sync.wait_ge(io, 16 * (1 + len(STORES)))

        @block.gpsimd
        def _(gpsimd):
            gpsimd.load_library(mlp)
            gpsimd.wait_ge(io, 16)  # idx in SBUF
            # async queues first (issue returns fast), inline queue-0 last
            # (it blocks the engine for its whole gen).
            order = [k for k, g in enumerate(GATHERS) if g[3] != 0] + [
                k for k, g in enumerate(GATHERS) if g[3] == 0
            ]
            for k in order:
                cap, slot0, wb, qn = GATHERS[k]
                b0, bw = slot0 // P, cap // P
                in_ap = twin[wb : wb + 32768, :]
                d3 = dst[:, b0 * E : (b0 + bw) * E].rearrange("p (b e) -> p b e", e=E)
                gpsimd.dma_gather(
                    d3,
                    in_ap,
                    idxs[:, slot0 // 16 : (slot0 + cap) // 16],
                    cap,
                    cap,
                    E,
                    queue_num=qn,
                ).then_inc(sem_of_gather[k], 16)

    nc.finalize()
    if CULL_UNUSED_ENGINES:
        _cull_unused_engines(nc, mybir)
    _hoist_library_load(nc, mybir)
    _CACHED_NC = nc
    return nc


def _hoist_library_load(nc, mybir):
    """Move the Pool library-reload pseudo-instruction from the start of the
    gpsimd block into the entry block, after the Pool const-pool memsets but
    before the Pool-side block barrier. The Q7 reload (~8.8us) runs async to
    the sequencer, so hoisting it past the barrier exchange starts the load
    ~0.8us earlier without delaying the other engines' block entry."""
    import concourse.bass_isa as bass_isa

    entry = nc.main_func.blocks[0]
    reload_inst = None
    for b in nc.main_func.blocks[1:]:
        for inst in b.instructions:
            if isinstance(inst, bass_isa.InstPseudoReloadLibraryIndex):
                reload_inst = inst
                break
        if reload_inst is not None:
            if reload_inst.sync_info is not None:
                assert not reload_inst.sync_info.on_wait
            b.instructions.remove(reload_inst)
            break
    assert reload_inst is not None
    # insert after the last Pool InstMemset in the entry block
    pos = 0
    for i, inst in enumerate(entry.instructions):
        if (
            isinstance(inst, mybir.InstMemset)
            and inst.engine == mybir.EngineType.Pool
        ):
            pos = i + 1
    entry.instructions.insert(pos, reload_inst)


# Drop the Tensor/Vector/Scalar engines from the compiled module: the kernel
# only uses Pool (gpsimd) and SP (sync), but Bacc emits entry/exit barrier
# instructions for all five engines, and the NEFF then carries five engine
# queues whose NRT launch/teardown handshakes sit on the critical path.
# Removing the three idle engines' barrier legs (and shrinking the Pool-side
# barrier counts from 4 peers to 1) trims the fixed preamble/teardown cost.
CULL_UNUSED_ENGINES = True


def _cull_unused_engines(nc, mybir):
    cull = {
        mybir.EngineType.PE,
        mybir.EngineType.DVE,
        mybir.EngineType.Activation,
    }
    n_peers = 1  # SP is the only remaining non-Pool engine
    for b in nc.main_func.blocks:
        kept = [
            inst for inst in b.instructions if getattr(inst, "engine", None) not in cull
        ]
        if len(kept) != len(b.instructions):
            b.instructions[:] = kept
        for inst in kept:
            si = inst.sync_info
            if si is None:
                continue
            for w in si.on_wait:
                if "barrier" in (w.ant_name or "") and w.wait_value == 4:
                    w.wait_value = n_peers
            for u in si.on_update:
                if "barrier" in (u.ant_name or "") and u.update_value == 4:
                    u.update_value = n_peers


def _shard_inputs(proc_emb, med_emb, chart_emb, concept, token_type):
    """Returns (in_maps, plans, tables) with per-core slot bookkeeping."""
    import ml_dtypes

    tables = np.ascontiguousarray(
        np.concatenate(
            [
                np.asarray(proc_emb, dtype=np.float32),
                np.asarray(med_emb, dtype=np.float32),
                np.asarray(chart_emb, dtype=np.float32),
            ],
            axis=0,
        )
    )
    # bf16 table for the device: halves every gathered/stored byte. Truncation
    # (>>16) costs <= 2^-8 relative error - far inside the 2e-2 budget.
    tables_bf = (tables.view(np.uint32) >> 16).astype(np.uint16).view(
        ml_dtypes.bfloat16
    )
    tt = np.asarray(token_type).reshape(-1).astype(np.int64)
    cc = np.asarray(concept).reshape(-1).astype(np.int64)
    typed = (tt >= 1) & (tt <= 3)
    toks_all = np.where(typed)[0]  # global token ids with a real lookup
    eff = cc[toks_all] + (tt[toks_all] - 1) * V  # their table rows

    core_of = eff // RSPAN
    local = eff - core_of * RSPAN

    in_maps = []
    plans = []  # per core: (tokens, dram_rows, overflow_tokens, overflow_rows)
    for c in range(NCORES):
        base = c * RSPAN
        sl = tables_bf[base : min(base + TWLEN, NROWS)]
        if sl.shape[0] < TWLEN:
            sl = np.concatenate(
                [sl, np.zeros((TWLEN - sl.shape[0], E), ml_dtypes.bfloat16)]
            )
        twin = np.ascontiguousarray(sl)

        sel = np.where(core_of == c)[0]
        order = sel[np.argsort(local[sel], kind="stable")]
        lrows = local[order]  # ascending
        gtoks = toks_all[order]
        n = len(lrows)

        # Window-B gather takes the largest CAP_B rows (they must be >= 5504
        # to fit window B; everything >= 32768 MUST land there). Rest goes
        # ascending into the three window-A gathers (< 32768 required),
        # filled sequentially. Count-overflow tails are gathered on the host.
        ovf_toks, ovf_rows = [], []
        n_hi = int(n - np.searchsorted(lrows, 32768))  # rows >= 32768
        if n_hi > CAP_B:  # window-B capacity overflow -> host
            spill = n_hi - CAP_B
            ovf_toks.extend(gtoks[n - spill :].tolist())
            ovf_rows.extend(lrows[n - spill :].tolist())
            lrows, gtoks, n = lrows[:-spill], gtoks[:-spill], n - spill
            n_hi = CAP_B
        # fill window B up to CAP_B with the largest remaining rows >= WB_BASE
        n_b = min(CAP_B, n - int(np.searchsorted(lrows, WB_BASE)))
        n_a = n - n_b
        if n_a > sum(CAPS_A):  # window-A capacity overflow -> host
            spill = n_a - sum(CAPS_A)
            # drop the largest window-A rows to the host
            ovf_toks.extend(gtoks[n_a - spill : n_a].tolist())
            ovf_rows.extend(lrows[n_a - spill : n_a].tolist())
            lrows = np.concatenate([lrows[: n_a - spill], lrows[n_a:]])
            gtoks = np.concatenate([gtoks[: n_a - spill], gtoks[n_a:]])
            n, n_a = n - spill, sum(CAPS_A)

        # per-gather (lo, hi) in the sorted row list: the window-B gather
        # takes the tail [n_a:n]; window-A gathers fill sequentially.
        spans = []
        lo = 0
        for cap, _s, wb, _q in GATHERS:
            if wb == WB_BASE:
                spans.append((n_a, n))
            else:
                hi = min(lo + cap, n_a)
                spans.append((lo, hi))
                lo = hi
        idx16 = np.zeros((16, SUMCAP // 16), dtype=np.int16)
        tok_list, row_list = [], []
        for (cap, slot0, wb, _q), (lo, hi) in zip(GATHERS, spans):
            wrows, wtoks = lrows[lo:hi], gtoks[lo:hi]
            cnt = len(wrows)
            vals = np.zeros(cap, dtype=np.int16)
            vals[:cnt] = (wrows - wb).astype(np.int16)  # pad 0 = benign row
            idx16[:, slot0 // 16 : (slot0 + cap) // 16] = (
                vals.reshape(cap // 16, 16).T
            )
            j = np.arange(cnt)
            row_list.append((j % P) * NB + slot0 // P + j // P)
            tok_list.append(wtoks)

        in_maps.append(
            {"twin": twin, "idx": np.ascontiguousarray(np.tile(idx16, (8, 1)))}
        )
        plans.append(
            (
                np.concatenate(tok_list),
                np.concatenate(row_list),
                np.array(ovf_toks, dtype=np.int64),
                np.array(ovf_rows, dtype=np.int64) + base,
            )
        )

    return in_maps, plans, tables


def _run(in_maps, trace=False):
    from concourse.bass_utils import run_bass_kernel_spmd

    nc = _build_bass()
    return run_bass_kernel_spmd(nc, in_maps, list(range(NCORES)), trace=trace)


def _assemble(results, plans, tables):
    out = np.zeros((N_TOK, E), dtype=np.float32)
    for c in range(NCORES):
        toks, drows, ovf_toks, ovf_rows = plans[c]
        if len(toks):
            out[toks] = results[c]["out"][drows].astype(np.float32)
        if len(ovf_toks):
            out[ovf_toks] = tables[ovf_rows]
    return out.reshape(B, S, E)


def kernel(proc_emb, med_emb, chart_emb, concept, token_type):
    in_maps, plans, tables = _shard_inputs(
        proc_emb, med_emb, chart_emb, concept, token_type
    )
    res = _run(in_maps, trace=False)
    return _assemble(res.results, plans, tables)
